# revision 4
# baseline (speedup 1.0000x reference)
"""CrossCorrelLoss kernel for Trainium2 (8 NeuronCores, data-parallel over batch).

Math: the reference normalizes x over dims (0,1) (global mean / unbiased std per
channel), computes per-batch gram matrices of the normalized data, means over
batch, gathers tril entries and compares against cross_correl_real. Because the
normalization stats are global, everything collapses to the raw second-moment
matrix of the flattened (B*T, N) data:
    G = X^T X,  S1 = column sums of X,  M = B*T
    mu = S1/M,  var = (diag(G) - M mu^2)/(M-1)
    C[i,j] = (G[i,j]/M - mu_i mu_j) / (sd_i sd_j)
    loss = sum |C[tril] - cross_correl_real| / 10

Each of the 8 cores computes partial G (tril row-blocks) and S1 over its
1/8 batch shard (8192x321); the host sums the 8 partials in float64 and does
the tiny 321x321 finalization.

Per-core pipeline (all data movement via SWDGE / gpsimd -- HWDGE measures only
~8.6 GB/s in this environment and was the old kernel's 1.23 ms bottleneck,
SWDGE sustains ~450+ GB/s):
- 16 chunked gpsimd cast-DMAs load the fp32 shard directly as bf16 into SBUF
  (inline fp32->bf16 cast, RNE, verified on HW); partition p holds 64
  consecutive rows. No scalar-engine cast stage.
- PE accumulates three tril row-blocks per 128-row step (small block first --
  stream cost is w0 + (w0+w1) + N, so the narrow block goes on the short
  stream):
    rows   0:66  x cols 0:66   -> psum0   (stream 66)
    rows  66:194 x cols 0:194  -> psum1   (stream 194)
    rows 194:321 x cols 0:321  -> psum2   (stream 321)
  581 streamed columns per step vs 705 for the 128/128/65 split and 963 for
  the naive full-G version.
- The block-2 stationary operand is 127 data cols + a column of ones (SBUF
  col 321, memset once), exactly filling 128 stationary columns: psum2
  partition 127 accumulates the column sums, so S1 comes out of the PE for
  free and the vector engine does no reductions.
- Vector engine only copies PSUM->SBUF at the end; one SWDGE DMA writes the
  ~298 KB result (G blocks + S1) back to DRAM.

bf16 input rounding with fp32 PSUM accumulation perturbs the final loss by
~2e-6 relative (verified against the fp32 reference on CPU); measured end to
end ~1.6e-5 including the reference's own fp32 run-to-run wobble.

build_nc(kreps>1) repeats the pipeline (double-buffered input / PSUM) purely
for slope-based wall-clock timing; the production path is kreps=1.
"""

import contextlib

import numpy as np

import concourse.bass as bass
import concourse.mybir as mybir
from concourse.bass_utils import run_bass_kernel_spmd

B, T, N = 128, 512, 321
NCORES = 8
M_TOTAL = B * T
M_CORE = M_TOTAL // NCORES   # 8192 rows per core
P = 128                      # SBUF partitions
RPP = M_CORE // P            # 64 rows per partition
NCHUNK = 16
RC = RPP // NCHUNK           # 4 rows per partition per chunk

NPAD = N + 1                 # 322: data cols 0..320, ones col 321
W0 = 66                      # block widths: streams are W0, W0+W1, N per step
W1 = 128                     # (small block first minimizes total streamed cols)
W2B = N - W0 - W1            # 127 data rows in block 2 + the ones col = 128
OUT_W = W0 + (W0 + W1) + N   # 66 + 194 + 321 = 581

_NC_CACHE = {}


def build_nc(kreps=1, psum_db=False):
    f32 = mybir.dt.float32
    bf16 = mybir.dt.bfloat16

    nc = bass.Bass()
    x = nc.declare_dram_parameter("x", [M_CORE, N], f32, isOutput=False)
    o_out = nc.declare_dram_parameter("o", [P, OUT_W], f32, isOutput=True)

    xv = x.rearrange("(p r) n -> p r n", p=P, r=RPP)
    nbuf = 2 if kreps > 1 else 1
    npsets = 2 if (psum_db and kreps > 1) else 1

    with contextlib.ExitStack() as ctx:
        xbs = [
            ctx.enter_context(nc.sbuf_tensor(f"xb{i}", [P, RPP, NPAD], bf16))
            for i in range(nbuf)
        ]
        out_t = ctx.enter_context(nc.sbuf_tensor("out_t", [P, OUT_W], f32))
        psets = [
            (
                ctx.enter_context(nc.psum_tensor(f"ps0_{i}", [P, W0], f32)),
                ctx.enter_context(nc.psum_tensor(f"ps1_{i}", [P, W0 + W1], f32)),
                ctx.enter_context(nc.psum_tensor(f"ps2_{i}", [P, N], f32)),
            )
            for i in range(npsets)
        ]
        dma_sems = [
            ctx.enter_context(nc.semaphore(f"dma_sem{c}")) for c in range(NCHUNK)
        ]
        init_sem = ctx.enter_context(nc.semaphore("init_sem"))
        pe_sem = ctx.enter_context(nc.semaphore("pe_sem"))
        dve_sem = ctx.enter_context(nc.semaphore("dve_sem"))
        odma_sem = ctx.enter_context(nc.semaphore("odma_sem"))
        block = ctx.enter_context(nc.Block())

        @block.gpsimd
        def _(gp):
            # iteration k's output DMA is issued after iteration k+1's inputs
            # so its dve wait doesn't stall the input pipeline
            for k in range(kreps):
                xb = xbs[k % nbuf]
                if k >= nbuf:
                    # don't overwrite a buffer the PE is still reading
                    gp.wait_ge(pe_sem, k - nbuf + 1)
                for c in range(NCHUNK):
                    gp.dma_start(
                        xb[:, c * RC : (c + 1) * RC, 0:N],
                        xv[:, c * RC : (c + 1) * RC, :],
                    ).then_inc(dma_sems[c], 16)
                if k >= 1:
                    gp.wait_ge(dve_sem, k)
                    gp.dma_start(o_out[:], out_t[:]).then_inc(odma_sem, 16)
            gp.wait_ge(dve_sem, kreps)
            gp.dma_start(o_out[:], out_t[:]).then_inc(odma_sem, 16)
            gp.wait_ge(odma_sem, 16 * kreps)

        @block.tensor
        def _(te):
            te.wait_ge(init_sem, 1)
            for k in range(kreps):
                xb = xbs[k % nbuf]
                ps0, ps1, ps2 = psets[k % npsets]
                if k >= npsets:
                    # PSUM reuse: wait until DVE copied the prior iteration
                    # using this set (concurrent PE write + DVE read of PSUM
                    # is a hard fault)
                    te.wait_ge(dve_sem, k - npsets + 1)
                for c in range(NCHUNK):
                    te.wait_ge(dma_sems[c], 16 * (k + 1))
                    for r in range(RC):
                        row = c * RC + r
                        first = row == 0
                        last = row == RPP - 1
                        te.matmul(
                            ps0[:W0, :],
                            xb[:, row, 0:W0],
                            xb[:, row, 0:W0],
                            start=first,
                            stop=last,
                        )
                        te.matmul(
                            ps1[:, :],
                            xb[:, row, W0 : W0 + W1],
                            xb[:, row, 0 : W0 + W1],
                            start=first,
                            stop=last,
                        )
                        # stationary = 127 data cols + the ones col (col 321):
                        # output partitions 0..126 are G rows 194..320,
                        # partition 127 accumulates the column sums S1
                        mm = te.matmul(
                            ps2[:, :],
                            xb[:, row, W0 + W1 : NPAD],
                            xb[:, row, 0:N],
                            start=first,
                            stop=last,
                        )
                        if last:
                            mm.then_inc(pe_sem, 1)

        @block.vector
        def _(ve):
            for i in range(nbuf):
                ve.memset(xbs[i][:, :, N : N + 1], 1.0)
            # partitions 66:128 of the ps0 region are never written; memset
            # (partition-start must be 0/32/64/96 -> start at 64, the ps0
            # copy on the same DVE queue overwrites partitions 64-65)
            ve.memset(out_t[64:, 0:W0], 0.0).then_inc(init_sem, 1)
            for k in range(kreps):
                ps0, ps1, ps2 = psets[k % npsets]
                if k > 0:
                    ve.wait_ge(odma_sem, 16 * k)
                ve.wait_ge(pe_sem, k + 1)
                ve.tensor_copy(out_t[:W0, 0:W0], ps0[:W0, :])
                ve.tensor_copy(out_t[:, W0 : 2 * W0 + W1], ps1[:, :])
                ve.tensor_copy(
                    out_t[:, 2 * W0 + W1 : OUT_W], ps2[:, :]
                ).then_inc(dve_sem, 1)

    return nc


def _get_nc(kreps=1):
    if kreps not in _NC_CACHE:
        _NC_CACHE[kreps] = build_nc(kreps)
    return _NC_CACHE[kreps]


def _finalize(o_parts, cross_correl_real):
    G = np.zeros((N, N), np.float64)
    S1 = np.zeros((N,), np.float64)
    for o in o_parts:
        o = np.asarray(o, dtype=np.float64)
        G[0:W0, 0:W0] += o[:W0, 0:W0]
        G[W0 : W0 + W1, 0 : W0 + W1] += o[:, W0 : 2 * W0 + W1]
        G[W0 + W1 : N, 0:N] += o[:W2B, 2 * W0 + W1 : OUT_W]
        S1 += o[W2B, 2 * W0 + W1 : OUT_W]
    # symmetrize: only the lower-tril row-blocks were computed
    iu = np.triu_indices(N, 1)
    G[iu] = G.T[iu]
    M = float(M_TOTAL)
    mu = S1 / M
    var = (np.diag(G) - M * mu * mu) / (M - 1.0)
    sd = np.sqrt(var)
    C = (G / M - np.outer(mu, mu)) / np.outer(sd, sd)
    i0, i1 = np.tril_indices(N)
    loss = np.abs(C[i0, i1] - cross_correl_real.astype(np.float64)).sum() / 10.0
    return np.float32(loss)


def kernel(x_fake, cross_correl_real):
    nc = _get_nc(1)
    x = np.ascontiguousarray(np.asarray(x_fake, dtype=np.float32)).reshape(B, T, N)
    bs = B // NCORES
    in_maps = [
        {"x": np.ascontiguousarray(x[i * bs : (i + 1) * bs].reshape(M_CORE, N))}
        for i in range(NCORES)
    ]
    res = run_bass_kernel_spmd(nc, in_maps, list(range(NCORES))).results
    return _finalize([r["o"] for r in res], np.asarray(cross_correl_real))
